# revision 2
# baseline (speedup 1.0000x reference)
"""Trainium2 Bass kernel v2 for nn_CriterionLP.

Two-path reduction, fp8-e4m3 DoubleRow matmuls (2 cols/cycle on PE):
  * Path A (exact): local support blocks [0, NA_BLK) in [anchor x support]
    orientation; DVE segmented tensor_reduce max from PSUM.
  * Path C (LSE p=4): local blocks [NA_BLK, 32) in transposed
    [support x anchor] orientation; ACT computes exp(80*sim - 80) -> bf16
    SBUF in the PSUM-drain pass, then PE one-hot ones-matmuls accumulate
    per-block sums S into a [nC, 512] PSUM tile; host m_hat = 1 + ln(S)/80.
    Block max via sharpened LSE: rel loss err ~1e-3 << 2e-2 tolerance.
  * Diagonal band (own identity blocks, rotated to local anchor tiles 0..3):
    dedicated [128, 512] psum tiles + DVE min-reduce (exact pos).
Anchors rotated by 512c per core so the program is SPMD-uniform.
"""

import numpy as np
import ml_dtypes

B = 4096
C = 128
TOPK = 8
K_INST = 16
P_IDS = B // K_INST
BLK = K_INST * TOPK            # 128 support cols per identity block
TEMP = 0.05
EPS = 1e-6

N_CORES = 8
S_LOC = B * TOPK // N_CORES    # 4096 support cols per core
NBLK_LOC = S_LOC // BLK        # 32 local blocks
A_ROT = B // N_CORES           # 512 anchor rotation per core
ATILE = 128
N_ATILES = B // ATILE          # 32
BPT = ATILE // K_INST          # 8 own-band blocks per anchor tile

NA_BLK = 16                    # local blocks on exact path A
NC_BLK = NBLK_LOC - NA_BLK     # blocks on LSE path C
NGRP = 8                       # anchor groups of 512 for path C
GRP = B // NGRP
P_LSE = 4                      # LSE sharpening: temp T/p -> scale 80
SCALE = P_LSE / TEMP           # 80

_CACHE = {}


def _build_program():
    import concourse.tile as tile
    from concourse import bacc, mybir
    from concourse.bass import ds, ts

    nc = bacc.Bacc(
        "TRN2", target_bir_lowering=False, debug=False, num_devices=N_CORES
    )
    f32 = mybir.dt.float32
    bf16 = mybir.dt.bfloat16
    f8 = mybir.dt.float8e4
    X = mybir.AxisListType.X
    DR = mybir.MatmulPerfMode.DoubleRow

    ft = nc.dram_tensor("ft", [C, B], bf16, kind="ExternalInput").ap()
    st = nc.dram_tensor("st", [C, S_LOC], bf16, kind="ExternalInput").ap()
    bmax = nc.dram_tensor("bmax", [B, NA_BLK], f32, kind="ExternalOutput").ap()
    bmin = nc.dram_tensor("bmin", [A_ROT, BPT], f32, kind="ExternalOutput").ap()
    lsum = nc.dram_tensor("lsum", [NGRP, NC_BLK, GRP], f32, kind="ExternalOutput").ap()

    with tile.TileContext(nc) as tc:
        with (
            tc.tile_pool(name="inp", bufs=1) as inp,
            tc.tile_pool(name="res", bufs=4) as resp,
            tc.tile_pool(name="ec", bufs=3) as ecp,
            tc.tile_pool(name="lout", bufs=2) as loutp,
            tc.tile_pool(name="psa", bufs=2, space="PSUM") as ppa,   # 2x[128,1024]
            tc.tile_pool(name="psc", bufs=3, space="PSUM") as ppc,   # 3x[128,512]
            tc.tile_pool(name="psr", bufs=1, space="PSUM") as ppr,   # 1x[<=32,512]
        ):
            ft_r = inp.tile([C, B], bf16)
            st_r = inp.tile([C, S_LOC], bf16)
            oh = inp.tile([C, NC_BLK, NC_BLK], bf16)
            nbias = inp.tile([C, 1], f32)
            scratch = inp.tile([C, 1], f32)

            # input DMA: first anchor tile pieces first
            nc.sync.dma_start(ft_r[:, ts(0, 512)], ft[:, ts(0, 512)])
            for q in range(4):
                nc.sync.dma_start(
                    st_r[:, ts(q, S_LOC // 4)], st[:, ts(q, S_LOC // 4)]
                )
            for q in range(1, 8):
                nc.sync.dma_start(ft_r[:, ts(q, 512)], ft[:, ts(q, 512)])

            nc.vector.memset(oh[:], 0.0)
            for b in range(NC_BLK):
                nc.vector.memset(oh[:, b, ds(b, 1)], 1.0)
            nc.vector.memset(nbias[:], -float(SCALE))
            # preload the Exp LUT before the pipeline needs it
            nc.scalar.activation(
                scratch[:], nbias[:], mybir.ActivationFunctionType.Exp,
                bias=nbias[:], scale=0.0,
            )

            # PE HAM warm-up (~3.5us of dummy matmuls during input DMA)
            warm = inp.tile([C, 512], bf16)
            nc.vector.memset(warm[:], 0.0)
            ps_w = ppa.tile([ATILE, 1024], f32, tag="psa")
            for i in range(14):
                nc.tensor.matmul(
                    ps_w[:, ds(512 * (i % 2), 512)],
                    warm[:, 0:ATILE],
                    warm[:],
                    start=True, stop=True,
                )

            # ---- interleaved A/C emission: one A half-chunk then two C
            # block-units, so the PE always has an independent matmul ready
            # while ACT/DVE drain the other path's PSUM tiles. ------------
            a_state = {}
            d_state = {}

            def a_unit(a, h):
                ps = ppa.tile([ATILE, 1024], f32, tag="psa")
                for j in range(2):
                    nc.tensor.matmul(
                        ps[:, ts(j, 512)],
                        ft_r[:, ts(a, ATILE)],
                        st_r[:, ds(1024 * h + 512 * j, 512)],
                        start=True, stop=True,
                    )
                if h == 0:
                    a_state[a] = resp.tile([ATILE, 16], f32, tag="res", name=f"res{a}")
                res = a_state[a]
                nc.vector.tensor_reduce(
                    res[:, ds(8 * h, 8)],
                    ps[:].rearrange("p (b x) -> p b x", x=BLK),
                    axis=X, op=mybir.AluOpType.max,
                )
                if h == 1:
                    nc.sync.dma_start(bmax[ts(a, ATILE), :], res[:])
                if a < 2 and h == a:
                    # own band of anchor tile a coincides with this chunk
                    mres = resp.tile([ATILE, 8], f32, tag="mres", name=f"mres{a}")
                    nc.vector.tensor_reduce(
                        mres[:],
                        ps[:].rearrange("p (b x) -> p b x", x=BLK),
                        axis=X, op=mybir.AluOpType.min,
                    )
                    nc.sync.dma_start(bmin[ts(a, ATILE), :], mres[:])

            def d_unit(a, h):
                # own band of anchor tile a<4: cols [1024a, 1024a+1024)
                psd = ppc.tile([ATILE, 512], f32, tag="psc")
                nc.tensor.matmul(
                    psd[:],
                    ft_r[:, ts(a, ATILE)],
                    st_r[:, ds(1024 * a + 512 * h, 512)],
                    start=True, stop=True,
                )
                if h == 0:
                    d_state[a] = resp.tile([ATILE, 8], f32, tag="mres", name=f"mres{a}")
                mres = d_state[a]
                nc.vector.tensor_reduce(
                    mres[:, ds(4 * h, 4)],
                    psd[:].rearrange("p (b x) -> p b x", x=BLK),
                    axis=X, op=mybir.AluOpType.min,
                )
                if h == 1:
                    nc.sync.dma_start(bmin[ts(a, ATILE), :], mres[:])

            c_state = {}

            def c_unit(g, bi):
                if bi == 0:
                    c_state[g] = ppr.tile([NC_BLK, 512], f32, tag="psr", name=f"psr{g}")
                psr = c_state[g]
                if bi % 4 == 0:
                    c_state[(g, "ec")] = ecp.tile(
                        [ATILE, 4, 512], bf16, tag="ec", name=f"ec{g}_{bi}"
                    )
                ec = c_state[(g, "ec")]
                b = NA_BLK + bi
                ps = ppc.tile([ATILE, 512], f32, tag="psc")
                nc.tensor.matmul(
                    ps[:],
                    st_r[:, ds(BLK * b, BLK)],
                    ft_r[:, ts(g, GRP)],
                    start=True, stop=True,
                )
                nc.scalar.activation(
                    ec[:, bi % 4, :], ps[:], mybir.ActivationFunctionType.Exp,
                    bias=nbias[:], scale=float(SCALE),
                )
                if bi % 4 == 3:
                    # one sem gates 4 back-to-back reduce matmuls
                    for k in range(4):
                        bj = bi - 3 + k
                        nc.tensor.matmul(
                            psr[:], oh[:, bj, :], ec[:, k, :],
                            start=(bj == 0), stop=(bj == NC_BLK - 1),
                        )
                if bi == NC_BLK - 1:
                    lo = loutp.tile([NC_BLK, 512], f32)
                    if g % 2 == 0:
                        nc.scalar.copy(lo[:], psr[:])
                    else:
                        nc.vector.tensor_copy(lo[:], psr[:])
                    nc.sync.dma_start(lsum[g], lo[:])

            # unit streams: 64 A units (+16 diag), 128 C units
            a_units = [(a, h) for a in range(N_ATILES) for h in range(2)]
            d_units = [(a, h) for a in range(2, 4) for h in range(2)]
            c_units = [(g, bi) for g in range(NGRP) for bi in range(NC_BLK)]
            ai = ci = di = 0
            step = 0
            while ai < len(a_units) or ci < len(c_units) or di < len(d_units):
                if ai < len(a_units):
                    a_unit(*a_units[ai]); ai += 1
                if di < len(d_units) and step % 8 == 3:
                    d_unit(*d_units[di]); di += 1
                for _ in range(2):
                    if ci < len(c_units):
                        c_unit(*c_units[ci]); ci += 1
                step += 1

    nc.compile()
    return nc


def _get_program():
    if "nc" not in _CACHE:
        _CACHE["nc"] = _build_program()
    return _CACHE["nc"]


def _pack8(x):
    """[N, C] f32 -> [C, N] bf16."""
    return np.ascontiguousarray(x.T).astype(ml_dtypes.bfloat16)


def _make_in_maps(feats, feats_s):
    fs = feats_s.reshape(B * TOPK, C)
    in_maps = []
    for c in range(N_CORES):
        ftc = _pack8(np.roll(feats, -A_ROT * c, axis=0))
        stc = _pack8(fs[S_LOC * c : S_LOC * (c + 1)])
        in_maps.append({"ft": ftc, "st": stc})
    return in_maps


def run_device(feats, feats_s, trace=False, tmpdir=None):
    from concourse.bass_utils import run_bass_kernel_spmd

    nc = _get_program()
    in_maps = _make_in_maps(feats, feats_s)
    kw = {}
    if trace:
        kw = dict(trace=True, tmpdir=tmpdir)
    r = run_bass_kernel_spmd(nc, in_maps, list(range(N_CORES)), **kw)

    blk_smax = np.empty((B, P_IDS), np.float64)
    pos_sim = np.empty((B,), np.float64)
    i = np.arange(A_ROT)
    for c in range(N_CORES):
        bm = np.asarray(r.results[c]["bmax"], np.float64)      # [B, NA_BLK]
        ls = np.asarray(r.results[c]["lsum"], np.float64)      # [NGRP, NC_BLK, GRP]
        m_hat = 1.0 + np.log(np.maximum(ls, 1e-300)) / SCALE   # [NGRP, NC_BLK, GRP]
        m_hat = m_hat.transpose(0, 2, 1).reshape(B, NC_BLK)    # local anchors x C-blocks
        loc = np.concatenate([bm, m_hat], axis=1)              # [B, 32] local blocks
        blk_smax[:, NBLK_LOC * c : NBLK_LOC * (c + 1)] = np.roll(
            loc, A_ROT * c, axis=0
        )
        mn = np.asarray(r.results[c]["bmin"])                  # [512, 8]
        pos_sim[A_ROT * c + i] = mn[i, (i // K_INST) % BPT]
    return blk_smax, pos_sim, r


def _loss_from_reductions(blk_smax, pos_sim, labels):
    e = np.exp(blk_smax / TEMP)
    own = e[np.arange(B), labels]
    neg = e.sum(axis=1) - own
    pos = np.exp(pos_sim / TEMP)
    loss = -np.log(pos / (pos + neg + EPS) + EPS)
    return np.float32(loss.mean())


def _numpy_fallback(feats, feats_s, labels):
    fs = feats_s.reshape(B * TOPK, C)
    sim = feats.astype(np.float64) @ fs.astype(np.float64).T
    e = np.exp(sim / TEMP).reshape(B, P_IDS, BLK)
    pos = e[np.arange(B), labels].min(axis=1)
    bm = e.max(axis=2)
    neg = bm.sum(axis=1) - bm[np.arange(B), labels]
    out = -np.log(pos / (pos + neg + EPS) + EPS)
    return np.float32(out.mean())


def kernel(**inputs):
    feats = np.ascontiguousarray(np.asarray(inputs["feats"], dtype=np.float32))
    feats_s = np.ascontiguousarray(np.asarray(inputs["feats_s"], dtype=np.float32))
    labels = np.asarray(inputs["labels"]).astype(np.int64)

    blk_smax, pos_sim, _ = run_device(feats, feats_s)

    if not np.array_equal(labels, np.arange(B, dtype=np.int64) // K_INST):
        return _numpy_fallback(feats, feats_s, labels)
    return _loss_from_reductions(blk_smax, pos_sim, labels)


# revision 3
# speedup vs baseline: 1.0542x; 1.0542x over previous
"""Trainium2 Bass kernel for nn_CriterionLP (hardest-pos/hardest-neg LP loss).

Math (reference):
    sim  = feats @ feats_s.reshape(B*TOPK, C).T          # [B, B*TOPK]
    blk  = exp(sim/T).reshape(B, P_IDS, K_INST*TOPK)
    pos  = min over own identity block                    # exp is monotone =>
    nmax = max over each identity block                   #   reduce raw sim, exp later
    loss = mean(-log(pos / (pos + sum_{j!=pid} nmax_j + eps) + eps))

Device strategy (8 NeuronCores, SPMD — one program, per-core data):
  * Shard the support dim: core c owns support columns [4096c, 4096(c+1))
    (= identity blocks [32c, 32c+32)); each core sees all B anchors.
  * Anchors are rotated by 512c per core so each core's "own block" diagonal
    band sits at local anchor tiles 0..3 / local blocks [8a, 8a+8) — the
    program is identical across cores.
  * Per core: [C=128 x 4096] @ [C=128 x 4096] fp32r matmuls into PSUM,
    DVE segmented tensor_reduce (max per 128-col identity block; min on the
    diagonal band only), DMA [4096, 32] block-max + [512, 8] band-min out.
  * Host: gather, exp at the [B, 256] level, assemble the scalar loss.
"""

import numpy as np

B = 4096
C = 128
TOPK = 8
K_INST = 16
P_IDS = B // K_INST            # 256 identity blocks
BLK = K_INST * TOPK            # 128 support cols per identity block
TEMP = 0.05
EPS = 1e-6

N_CORES = 8
S_LOC = B * TOPK // N_CORES    # 4096 support cols per core
NBLK_LOC = S_LOC // BLK        # 32 identity blocks per core
A_ROT = B // N_CORES           # 512: per-core anchor rotation
ATILE = 128                    # anchors per tile (partition dim)
N_ATILES = B // ATILE          # 32
BPT = ATILE // K_INST          # 8 own-band blocks per anchor tile

_CACHE = {}

# Greedy DVE/ACT load balance (measured ns per half-tile).
DVE_DIRECT = 2280.0   # tensor_reduce [128, 16x128] from PSUM
DVE_MIN = 1190.0      # extra band min reduce from PSUM
DVE_TREE = 1670.0     # 4 fp16 2x folds to width 8 (host finishes 8->1)
ACT_CAST = 2160.0     # PSUM f32 -> SBUF fp16 copy (measured)


def _schedule():
    """Static DVE/ACT assignment; must match between build and host gather."""
    direct_map = {}
    dve_load, act_load = 0.0, 0.0
    for a in range(N_ATILES):
        for h in range(2):
            diag = a < 4 and h == a // 2
            direct = diag or (dve_load + DVE_DIRECT <= act_load + ACT_CAST)
            if direct:
                dve_load += DVE_DIRECT + (DVE_MIN if diag else 0.0)
            else:
                act_load += ACT_CAST
                dve_load += DVE_TREE
            direct_map[(a, h)] = direct
    return direct_map


def _build_program():
    import concourse.tile as tile
    from concourse import bacc, mybir
    from concourse.bass import ds, ts

    nc = bacc.Bacc(
        "TRN2", target_bir_lowering=False, debug=False, num_devices=N_CORES
    )
    f32 = mybir.dt.float32
    f16 = mybir.dt.float16
    X = mybir.AxisListType.X

    ft = nc.dram_tensor("ft", [C, B], f16, kind="ExternalInput").ap()
    st = nc.dram_tensor("st", [C, S_LOC], f16, kind="ExternalInput").ap()
    bmax = nc.dram_tensor("bmax", [B, NBLK_LOC], f32, kind="ExternalOutput").ap()
    bmin = nc.dram_tensor("bmin", [A_ROT, BPT], f32, kind="ExternalOutput").ap()
    bm8 = nc.dram_tensor("bm8", [B, 2, 16, 8], f16, kind="ExternalOutput").ap()

    direct_map = _schedule()

    with tile.TileContext(nc) as tc:
        with (
            tc.tile_pool(name="inp", bufs=1) as inp,
            tc.tile_pool(name="res", bufs=4) as resp,
            tc.tile_pool(name="minres", bufs=2) as minp,
            tc.tile_pool(name="cast", bufs=7) as castp,
            tc.tile_pool(name="tree", bufs=6) as treep,
            tc.tile_pool(name="psum", bufs=2, space="PSUM") as pp,
        ):
            ft_r = inp.tile([C, B], f16)
            st_r = inp.tile([C, S_LOC], f16)
            # First anchor tile needs ft[:, 0:128] and st[:, 0:2048]; emit
            # those pieces first so the pipeline starts as soon as possible.
            nc.sync.dma_start(ft_r[:, ts(0, 512)], ft[:, ts(0, 512)])
            for q in range(4):
                nc.sync.dma_start(
                    st_r[:, ts(q, S_LOC // 4)], st[:, ts(q, S_LOC // 4)]
                )
            for q in range(1, 8):
                nc.sync.dma_start(ft_r[:, ts(q, 512)], ft[:, ts(q, 512)])

            # PE HAM warm-up: ~7us of back-to-back dummy matmuls during the
            # input-DMA window flips the clock gate to 8/8 (2.4 GHz) before
            # the real work starts; steady-state gaps are short enough to
            # stay warm after that. Reads an uninitialized scratch tile (no
            # deps -> scheduled first), writes the first PSUM slot.
            warm = inp.tile([C, 512], f16)
            nc.scalar.memzero(warm[:])
            ps_w = pp.tile([ATILE, 4 * 512], f32, tag="ps")
            for i in range(7):
                nc.tensor.matmul(
                    ps_w[:, ts(i % 4, 512)],
                    warm[:, 0:ATILE],
                    warm[:],
                    start=True,
                    stop=True,
                )

            for a in range(N_ATILES):
                for h in range(2):  # two PSUM halves of 2048 support cols
                    ps = pp.tile([ATILE, 4 * 512], f32, tag="ps")
                    for j in range(4):
                        nc.tensor.matmul(
                            ps[:, ts(j, 512)],
                            ft_r[:, ts(a, ATILE)],
                            st_r[:, ds(2048 * h + 512 * j, 512)],
                            start=True,
                            stop=True,
                        )
                    diag = a < 4 and h == a // 2
                    if direct_map[(a, h)]:
                        res = resp.tile([ATILE, 16], f32)
                        nc.vector.tensor_reduce(
                            res[:],
                            ps[:].rearrange("p (b x) -> p b x", x=BLK),
                            axis=X,
                            op=mybir.AluOpType.max,
                        )
                        nc.sync.dma_start(
                            bmax[ts(a, ATILE), ds(16 * h, 16)], res[:]
                        )
                        if diag:
                            # own-block band: local blocks [8a, 8a+8)
                            mres = minp.tile([ATILE, BPT], f32)
                            nc.vector.tensor_reduce(
                                mres[:],
                                ps[:, ds((a % 2) * 1024, 1024)].rearrange(
                                    "p (b x) -> p b x", x=BLK
                                ),
                                axis=X,
                                op=mybir.AluOpType.min,
                            )
                            nc.sync.dma_start(bmin[ts(a, ATILE), :], mres[:])
                    else:
                        s = castp.tile([ATILE, 16, BLK], f16)
                        nc.scalar.copy(s[:], ps[:].rearrange("p (b x) -> p b x", x=BLK))
                        # fp16 2x pairwise-max folds down to width 8;
                        # the host finishes the last 8->1 reduction.
                        cur = s
                        for w in (64, 32, 16, 8):
                            nxt = treep.tile([ATILE, 16, w], f16, tag=f"tree{w}")
                            nc.vector.tensor_tensor(
                                nxt[:],
                                cur[:, :, 0:w],
                                cur[:, :, w : 2 * w],
                                op=mybir.AluOpType.max,
                            )
                            cur = nxt
                        nc.sync.dma_start(bm8[ts(a, ATILE), h, :, :], cur[:])

    nc.compile()
    return nc


def _get_program():
    if "nc" not in _CACHE:
        _CACHE["nc"] = _build_program()
    return _CACHE["nc"]


def _make_in_maps(feats, feats_s):
    fs = feats_s.reshape(B * TOPK, C)
    in_maps = []
    for c in range(N_CORES):
        ftc = np.ascontiguousarray(np.roll(feats, -A_ROT * c, axis=0).T).astype(
            np.float16
        )
        stc = np.ascontiguousarray(fs[S_LOC * c : S_LOC * (c + 1)].T).astype(
            np.float16
        )
        in_maps.append({"ft": ftc, "st": stc})
    return in_maps


def run_device(feats, feats_s, trace=False, tmpdir=None):
    """Run the SPMD program; return (blk_smax [B, P_IDS], pos_sim [B], raw)."""
    from concourse.bass_utils import run_bass_kernel_spmd

    nc = _get_program()
    in_maps = _make_in_maps(feats, feats_s)
    kw = {}
    if trace:
        kw = dict(trace=True, tmpdir=tmpdir)
    r = run_bass_kernel_spmd(nc, in_maps, list(range(N_CORES)), **kw)

    direct_map = _schedule()
    blk_smax = np.empty((B, P_IDS), np.float64)
    pos_sim = np.empty((B,), np.float64)
    i = np.arange(A_ROT)
    for c in range(N_CORES):
        bm = np.array(r.results[c]["bmax"])    # [B, 32]; valid on direct halves
        bm8 = np.asarray(r.results[c]["bm8"])  # [B, 2, 16, 8] fp16 tree tops
        bm8 = bm8.astype(np.float32).max(axis=3)  # [B, 2, 16]
        for a in range(N_ATILES):
            for h in range(2):
                if not direct_map[(a, h)]:
                    bm[128 * a : 128 * (a + 1), 16 * h : 16 * (h + 1)] = bm8[
                        128 * a : 128 * (a + 1), h
                    ]
        blk_smax[:, NBLK_LOC * c : NBLK_LOC * (c + 1)] = np.roll(
            bm, A_ROT * c, axis=0
        )
        mn = np.asarray(r.results[c]["bmin"])  # [512, 8] band mins
        pos_sim[A_ROT * c + i] = mn[i, (i // K_INST) % BPT]
    return blk_smax, pos_sim, r


def _loss_from_reductions(blk_smax, pos_sim, labels):
    e = np.exp(blk_smax / TEMP)             # [B, P_IDS] block max of exp
    own = e[np.arange(B), labels]
    neg = e.sum(axis=1) - own
    pos = np.exp(pos_sim / TEMP)
    loss = -np.log(pos / (pos + neg + EPS) + EPS)
    return np.float32(loss.mean())


def _numpy_fallback(feats, feats_s, labels):
    # Exact mirror of the reference, host-only. Safety net for label
    # patterns other than arange(B)//K_INST (never produced by setup_inputs).
    fs = feats_s.reshape(B * TOPK, C)
    out = np.empty((B,), np.float64)
    sim = feats.astype(np.float64) @ fs.astype(np.float64).T
    e = np.exp(sim / TEMP).reshape(B, P_IDS, BLK)
    pos = e[np.arange(B), labels].min(axis=1)
    bm = e.max(axis=2)
    neg = bm.sum(axis=1) - bm[np.arange(B), labels]
    out = -np.log(pos / (pos + neg + EPS) + EPS)
    return np.float32(out.mean())


def kernel(**inputs):
    feats = np.ascontiguousarray(np.asarray(inputs["feats"], dtype=np.float32))
    feats_s = np.ascontiguousarray(np.asarray(inputs["feats_s"], dtype=np.float32))
    labels = np.asarray(inputs["labels"]).astype(np.int64)

    blk_smax, pos_sim, _ = run_device(feats, feats_s)

    if not np.array_equal(labels, np.arange(B, dtype=np.int64) // K_INST):
        return _numpy_fallback(feats, feats_s, labels)
    return _loss_from_reductions(blk_smax, pos_sim, labels)

